# revision 47
# baseline (speedup 1.0000x reference)
"""TRN2 Bass kernel for nn_Attention_17935783428543.

Reference computation (per batch b of 4):
  qkv = w_qkv @ X        (X = x[b] as [C=128, N=4096])
  per head h (4 heads, d=32): sim = (q_h * scale)^T k_h ; P = softmax(sim)
  y_h = P @ v_h^T ; out = w_out @ concat_h(y_h^T) + b_out

Sharding: 8 cores = 4 batches x 2 query-halves. Each core computes the full
attention for its batch restricted to 2048 query pixels (all 4096 keys), all
4 heads, including QKV projection and the output projection. No collectives.
The query-half assignment uses a host-side rotation of x's pixel axis so all
8 cores run the identical SPMD graph: queries are always columns 0:2048.

Per-core design (all matmuls bf16, PSUM f32). The kernel is ScalarE-bound:
softmax needs 33.5M exps/core at 1 elem/lane/cycle. Everything is organized
to keep one ACT exp instruction running at all times with minimal
per-instruction overhead:
  - sim^T tiles [j=128, i=512] per head, K=32 row-packed on the PE via
    tile_position. Head-sims form a flat stream packed 3-per-window into
    [128, 1536] PSUM tiles (3 banks); exp processes a whole window in one
    instruction (1536 is an ACT size sweet spot, ~1.44us back-to-back).
  - the window ring is double buffered (2x3 banks), so sims of window w+2
    run while window w+1's exp streams: ACT never waits on the sim WAR.
  - P@V via col-packed matmuls (M=32/head) accumulating y^T [4h*32d, i] in
    one PSUM bank. Softmax denominators: adjacent j-chunks' P are pair-summed
    on the (otherwise idle) VectorE, then a ones[128,32] matmul per pair
    accumulates row-sums broadcast to each head's 32-row block — halving the
    PE's sums stream; every deferred piece runs a window later than its
    producer so nothing sits on the ACT critical path.
  - per i-chunk epilogue: 1/sums via fast-approx reciprocal, y^T scaled to
    bf16, w_out^T projection (+bias via per-partition tensor_scalar add).
    The outproj PSUM tile borrows the sums bank (freed by the reciprocal);
    the epilogue is split across two window bodies so its PE work never
    sits in the FIFO ahead of the next window's sims.
  - QKV projections stream through the same PSUM ring as the sim windows,
    interleaved with the first windows so the exp stream starts as soon as
    x's first DMA chunk lands.
"""

import numpy as np
import ml_dtypes

import concourse.mybir as mybir
import concourse.tile as tile
from concourse import bacc
from concourse.bass_utils import run_bass_kernel_spmd

F32 = mybir.dt.float32
BF16 = mybir.dt.bfloat16
NPBF16 = ml_dtypes.bfloat16

B = 4
C = 128
HEADS = 4
D = 32
N = 4096          # pixels per batch (64*64)
NQ = 2048         # query pixels per core
SCALE = D ** -0.5
I_CHUNK = 512
J_CHUNK = 128
N_I = NQ // I_CHUNK     # 4
N_J = N // J_CHUNK      # 32

_NC_CACHE = {}


def _build_nc():
    nc = bacc.Bacc("TRN2", target_bir_lowering=False, debug=False, num_devices=8)

    x = nc.dram_tensor("x", [N // 512, C, 512], BF16, kind="ExternalInput").ap()
    wq = nc.dram_tensor("wq", [C, C], BF16, kind="ExternalInput").ap()
    wk = nc.dram_tensor("wk", [C, C], BF16, kind="ExternalInput").ap()
    wv = nc.dram_tensor("wv", [C, C], BF16, kind="ExternalInput").ap()
    wo = nc.dram_tensor("wo", [C, C], BF16, kind="ExternalInput").ap()
    bo = nc.dram_tensor("bo", [C, 1], F32, kind="ExternalInput").ap()
    out = nc.dram_tensor("out", [C, NQ], F32, kind="ExternalOutput").ap()

    with tile.TileContext(nc) as tc:
        with (
            tc.tile_pool(name="const", bufs=1) as cpool,
            tc.tile_pool(name="acts", bufs=1) as apool,
            tc.tile_pool(name="pt", bufs=4) as ptpool,
            tc.tile_pool(name="epi", bufs=2) as epool,
            tc.tile_pool(name="psum_ring", bufs=2, space="PSUM") as pring,
            tc.tile_pool(name="psum_acc", bufs=1, space="PSUM") as pacc,
        ):
            # ---- constants / weights ----
            wq_sb = cpool.tile([C, C], BF16, tag="wq")
            nc.sync.dma_start(wq_sb[:], wq)
            wk_sb = cpool.tile([C, C], BF16, tag="wk")
            nc.sync.dma_start(wk_sb[:], wk)
            wv_sb = cpool.tile([C, C], BF16, tag="wv")
            nc.sync.dma_start(wv_sb[:], wv)
            wo_sb = cpool.tile([C, C], BF16, tag="wo")
            nc.sync.dma_start(wo_sb[:], wo)
            bo_sb = cpool.tile([C, 1], F32, tag="bo")
            nc.sync.dma_start(bo_sb[:], bo)
            ones32 = cpool.tile([128, 32], BF16, tag="ones32")
            nc.vector.memset(ones32[:], 1.0)

            # warm the ACT exp table during the DMA prologue
            warm = cpool.tile([1, 1], F32, tag="warm")
            nc.vector.memset(warm[:], 0.0)
            nc.scalar.activation(warm[:], warm[:], mybir.ActivationFunctionType.Exp)

            # ---- x DMA (8 chunks so early projections can start early) ----
            x_sb = apool.tile([C, N], BF16, tag="x")
            for g in range(N // 512):
                nc.gpsimd.dma_start(x_sb[:, 512 * g : 512 * (g + 1)], x[g])

            q_all = apool.tile([C, NQ], BF16, tag="q")    # [4h*32c', i]
            k_all = apool.tile([C, N], BF16, tag="k")     # [4h*32c', j]
            vT_all = apool.tile([C, N], BF16, tag="vT")   # chunk J cols J*128.. : [j, 4h*32d]

            # ---- projection units, streamed through the psum ring ----
            def emit_proj_batch(units):
                width = 512 * len(units)
                ps = pring.tile([128, width], F32, tag="ring", name="proj_ps")
                for u, (kind, g) in enumerate(units):
                    sl = slice(512 * u, 512 * (u + 1))
                    if kind == "q":
                        nc.tensor.matmul(
                            ps[:, sl],
                            lhsT=wq_sb[:],
                            rhs=x_sb[:, 512 * g : 512 * (g + 1)],
                            start=True,
                            stop=True,
                        )
                        nc.vector.tensor_copy(q_all[:, 512 * g : 512 * (g + 1)], ps[:, sl])
                    elif kind == "k":
                        nc.tensor.matmul(
                            ps[:, sl],
                            lhsT=wk_sb[:],
                            rhs=x_sb[:, 512 * g : 512 * (g + 1)],
                            start=True,
                            stop=True,
                        )
                        nc.vector.tensor_copy(k_all[:, 512 * g : 512 * (g + 1)], ps[:, sl])
                    else:  # v^T: per 128-pixel chunk, out[j, o'] = x^T wv
                        for c4 in range(4):
                            J = 4 * g + c4
                            nc.tensor.matmul(
                                ps[:, 512 * u + 128 * c4 : 512 * u + 128 * (c4 + 1)],
                                lhsT=x_sb[:, 128 * J : 128 * (J + 1)],
                                rhs=wv_sb[:],
                                start=True,
                                stop=True,
                            )
                        nc.vector.tensor_copy(vT_all[:, 512 * g : 512 * (g + 1)], ps[:, sl])

            # ---- attention stream ----
            # deferral scheduler: actions run at the start of their due body,
            # in insertion order (bodies past the window list keep flushing)
            acc_tiles = {}
            pending = []  # (due_body, fn)

            def schedule(due, fn):
                pending.append((due, fn))

            def flush(w):
                i = 0
                while i < len(pending):
                    due, fn = pending[i]
                    if due <= w:
                        pending.pop(i)
                        fn()
                    else:
                        i += 1

            half_sums = {}  # (I, h) -> (pTw, col) of the even-J partner

            def emit_pv(slots, pTw, w):
                for k, (I, J, h) in enumerate(slots):
                    if I not in acc_tiles:
                        acc_tiles[I] = (
                            pacc.tile([128, I_CHUNK], F32, tag="yT", name="yT"),
                            pacc.tile([128, I_CHUNK], F32, tag="sums", name="sums"),
                        )
                    nc.tensor.matmul(
                        acc_tiles[I][0][32 * h : 32 * h + 32, :],
                        lhsT=vT_all[:, 128 * J + 32 * h : 128 * J + 32 * h + 32],
                        rhs=pTw[:, 512 * k : 512 * (k + 1)],
                        start=(J == 0),
                        stop=(J == N_J - 1),
                        tile_position=(0, 32 * h),
                    )
                # denominators: pair adjacent j-chunks' P on VectorE, then one
                # ones-matmul per pair (emitted one body later so the DVE add
                # is never on the PE/ACT critical path) — halves the PE's
                # sums-stream cost
                for k, (I, J, h) in enumerate(slots):
                    if J % 2 == 0:
                        half_sums[(I, h)] = (pTw, k)
                        continue
                    ppTw, pk = half_sums.pop((I, h))
                    pair = epool.tile([128, I_CHUNK], BF16, tag="pair", name="pair", bufs=4)
                    nc.vector.tensor_tensor(
                        pair[:],
                        ppTw[:, 512 * pk : 512 * (pk + 1)],
                        pTw[:, 512 * k : 512 * (k + 1)],
                        mybir.AluOpType.add,
                    )

                    def sums_mm(I=I, J=J, h=h, pair=pair):
                        nc.tensor.matmul(
                            acc_tiles[I][1][32 * h : 32 * h + 32, :],
                            lhsT=ones32[:],
                            rhs=pair[:],
                            start=(J == 1),
                            stop=(J == N_J - 1),
                            tile_position=(0, 32 * h),
                        )
                        if (J, h) == (N_J - 1, HEADS - 1):
                            ynorm = emit_epilogue1(I)
                            schedule(w + 1, lambda: emit_epilogue2(I, ynorm))

                    # the chunk's last pairs skip the one-window deferral:
                    # the epilogue (and the next chunk's accumulator banks)
                    # wait on them, and there is no later exp to protect
                    schedule(w if J == N_J - 1 else w + 1, sums_mm)

            def emit_epilogue1(I):
                yT, sums = acc_tiles.pop(I)
                s_r = epool.tile([128, I_CHUNK], F32, tag="s_r")
                nc.vector.reciprocal_approx_fast(s_r[:], sums[:])
                ynorm = epool.tile([128, I_CHUNK], BF16, tag="ynorm")
                nc.vector.tensor_tensor(ynorm[:], yT[:], s_r[:], mybir.AluOpType.mult)
                return ynorm

            def emit_epilogue2(I, ynorm):
                isl = slice(I_CHUNK * I, I_CHUNK * (I + 1))
                # outproj borrows the sums bank (already freed by the recip)
                op = pacc.tile([128, I_CHUNK], F32, tag="sums", name="op")
                nc.tensor.matmul(op[:], lhsT=wo_sb[:], rhs=ynorm[:], start=True, stop=True)
                out_sb = epool.tile([128, I_CHUNK], F32, tag="out_sb")
                nc.vector.tensor_scalar_add(out_sb[:], op[:], bo_sb[:, :])
                nc.sync.dma_start(out[:, isl], out_sb[:])

            def emit_window(slots, w):
                width = 512 * len(slots)
                simw = pring.tile([128, width], F32, tag="ring", name="simw")
                for k, (I, J, h) in enumerate(slots):
                    nc.tensor.matmul(
                        simw[:, 512 * k : 512 * (k + 1)],
                        lhsT=k_all[32 * h : 32 * h + 32, 128 * J : 128 * (J + 1)],
                        rhs=q_all[32 * h : 32 * h + 32, I_CHUNK * I : I_CHUNK * (I + 1)],
                        start=True,
                        stop=True,
                        tile_position=(32 * h, 0),
                    )
                pTw = ptpool.tile([128, width], BF16, tag="pT", name="pTw")
                nc.scalar.activation(
                    pTw[:], simw[:], mybir.ActivationFunctionType.Exp, scale=SCALE
                )
                flush(w)
                schedule(w + 1, lambda: emit_pv(slots, pTw, w + 1))

            # window list: per i-chunk, 42 windows of 3 head-sims + 1 of 2
            windows = []
            for I in range(N_I):
                slots = [(I, t // 4, t % 4) for t in range(N_J * HEADS)]
                windows.extend(
                    slots[3 * w : 3 * w + 3] for w in range((len(slots) + 2) // 3)
                )

            # projections: prime the ring with what window 0 needs, then
            # spread the rest as single units on a just-in-time schedule so
            # they never pile PE work into the ramp windows
            emit_proj_batch([("k", 0), ("q", 0)])
            emit_proj_batch([("k", 1), ("v", 0)])
            late_units = [
                ("v", 1), ("k", 2), ("v", 2), ("k", 3), ("v", 3), ("k", 4),
                ("v", 4), ("k", 5), ("v", 5), ("k", 6), ("v", 6), ("k", 7),
                ("v", 7), ("q", 1), ("q", 2), ("q", 3),
            ]
            for i, unit in enumerate(late_units):
                schedule(2 + 2 * i, lambda unit=unit: emit_proj_batch([unit]))
            for w, slots in enumerate(windows):
                emit_window(slots, w)
            w = len(windows)
            while pending:
                flush(w)
                w += 1

    nc.compile()
    return nc


def kernel(x, w_qkv, w_out, b_out, _trace=False):
    if "nc" not in _NC_CACHE:
        _NC_CACHE["nc"] = _build_nc()
    nc = _NC_CACHE["nc"]

    x = np.asarray(x, dtype=np.float32).reshape(B, C, N)
    w_qkv = np.asarray(w_qkv, dtype=np.float32)
    w_out = np.asarray(w_out, dtype=np.float32)
    b_out = np.asarray(b_out, dtype=np.float32)

    wq = np.ascontiguousarray(w_qkv[0:C].T).astype(NPBF16)
    wk = np.ascontiguousarray(w_qkv[C : 2 * C].T).astype(NPBF16)
    wv = np.ascontiguousarray(w_qkv[2 * C : 3 * C].T).astype(NPBF16)
    wo = np.ascontiguousarray(w_out.T).astype(NPBF16)
    bo = np.ascontiguousarray(b_out.reshape(C, 1))

    in_maps = []
    for core in range(8):
        b, half = core >> 1, core & 1
        xb = x[b]
        if half:
            xb = np.concatenate([xb[:, NQ:], xb[:, :NQ]], axis=1)
        xb_c = np.ascontiguousarray(
            xb.reshape(C, N // 512, 512).transpose(1, 0, 2)
        ).astype(NPBF16)
        in_maps.append(
            {
                "x": xb_c,
                "wq": wq,
                "wk": wk,
                "wv": wv,
                "wo": wo,
                "bo": bo,
            }
        )

    res = run_bass_kernel_spmd(nc, in_maps, list(range(8)), trace=_trace)

    full = np.empty((B, C, N), np.float32)
    for core in range(8):
        b, half = core >> 1, core & 1
        full[b][:, NQ * half : NQ * (half + 1)] = res.results[core]["out"]
    out = full.reshape(B, C, 64, 64)
    if _trace:
        return out, res
    return out


# revision 48
# speedup vs baseline: 1.0137x; 1.0137x over previous
"""TRN2 Bass kernel for nn_Attention_17935783428543.

Reference computation (per batch b of 4):
  qkv = w_qkv @ X        (X = x[b] as [C=128, N=4096])
  per head h (4 heads, d=32): sim = (q_h * scale)^T k_h ; P = softmax(sim)
  y_h = P @ v_h^T ; out = w_out @ concat_h(y_h^T) + b_out

Sharding: 8 cores = 4 batches x 2 query-halves. Each core computes the full
attention for its batch restricted to 2048 query pixels (all 4096 keys), all
4 heads, including QKV projection and the output projection. No collectives.
The query-half assignment uses a host-side rotation of x's pixel axis so all
8 cores run the identical SPMD graph: queries are always columns 0:2048.

Per-core design (all matmuls bf16, PSUM f32). The kernel is ScalarE-bound:
softmax needs 33.5M exps/core at 1 elem/lane/cycle. Everything is organized
to keep one ACT exp instruction running at all times with minimal
per-instruction overhead:
  - sim^T tiles [j=128, i=512] per head, K=32 row-packed on the PE via
    tile_position. Head-sims form a flat stream packed 3-per-window into
    [128, 1536] PSUM tiles (3 banks); exp processes a whole window in one
    instruction (1536 is an ACT size sweet spot, ~1.44us back-to-back).
  - the window ring is double buffered (2x3 banks), so sims of window w+2
    run while window w+1's exp streams: ACT never waits on the sim WAR.
  - P@V via col-packed matmuls (M=32/head) accumulating y^T [4h*32d, i] in
    one PSUM bank. Softmax denominators: adjacent j-chunks' P are pair-summed
    on the (otherwise idle) VectorE, then a ones[128,32] matmul per pair
    accumulates row-sums broadcast to each head's 32-row block — halving the
    PE's sums stream; every deferred piece runs a window later than its
    producer so nothing sits on the ACT critical path.
  - per i-chunk epilogue: 1/sums via fast-approx reciprocal, y^T scaled to
    bf16, w_out^T projection (+bias via per-partition tensor_scalar add).
    The outproj PSUM tile borrows the sums bank (freed by the reciprocal);
    the epilogue is split across two window bodies so its PE work never
    sits in the FIFO ahead of the next window's sims.
  - QKV projections stream through the same PSUM ring as the sim windows,
    interleaved with the first windows so the exp stream starts as soon as
    x's first DMA chunk lands.
"""

import numpy as np
import ml_dtypes

import concourse.mybir as mybir
import concourse.tile as tile
from concourse import bacc
from concourse.bass_utils import run_bass_kernel_spmd

F32 = mybir.dt.float32
BF16 = mybir.dt.bfloat16
NPBF16 = ml_dtypes.bfloat16

B = 4
C = 128
HEADS = 4
D = 32
N = 4096          # pixels per batch (64*64)
NQ = 2048         # query pixels per core
SCALE = D ** -0.5
I_CHUNK = 512
J_CHUNK = 128
N_I = NQ // I_CHUNK     # 4
N_J = N // J_CHUNK      # 32

_NC_CACHE = {}


def _build_nc():
    nc = bacc.Bacc("TRN2", target_bir_lowering=False, debug=False, num_devices=8)

    x = nc.dram_tensor("x", [N // 512, C, 512], BF16, kind="ExternalInput").ap()
    wq = nc.dram_tensor("wq", [C, C], BF16, kind="ExternalInput").ap()
    wk = nc.dram_tensor("wk", [C, C], BF16, kind="ExternalInput").ap()
    wv = nc.dram_tensor("wv", [C, C], BF16, kind="ExternalInput").ap()
    wo = nc.dram_tensor("wo", [C, C], BF16, kind="ExternalInput").ap()
    bo = nc.dram_tensor("bo", [C, 1], F32, kind="ExternalInput").ap()
    out = nc.dram_tensor("out", [C, NQ], F32, kind="ExternalOutput").ap()

    with tile.TileContext(nc) as tc:
        with (
            tc.tile_pool(name="const", bufs=1) as cpool,
            tc.tile_pool(name="acts", bufs=1) as apool,
            tc.tile_pool(name="pt", bufs=6) as ptpool,
            tc.tile_pool(name="epi", bufs=2) as epool,
            tc.tile_pool(name="psum_ring", bufs=2, space="PSUM") as pring,
            tc.tile_pool(name="psum_acc", bufs=1, space="PSUM") as pacc,
        ):
            # ---- constants / weights ----
            wq_sb = cpool.tile([C, C], BF16, tag="wq")
            nc.sync.dma_start(wq_sb[:], wq)
            wk_sb = cpool.tile([C, C], BF16, tag="wk")
            nc.sync.dma_start(wk_sb[:], wk)
            wv_sb = cpool.tile([C, C], BF16, tag="wv")
            nc.sync.dma_start(wv_sb[:], wv)
            wo_sb = cpool.tile([C, C], BF16, tag="wo")
            nc.sync.dma_start(wo_sb[:], wo)
            bo_sb = cpool.tile([C, 1], F32, tag="bo")
            nc.sync.dma_start(bo_sb[:], bo)
            ones32 = cpool.tile([128, 32], BF16, tag="ones32")
            nc.vector.memset(ones32[:], 1.0)

            # warm the ACT exp table during the DMA prologue
            warm = cpool.tile([1, 1], F32, tag="warm")
            nc.vector.memset(warm[:], 0.0)
            nc.scalar.activation(warm[:], warm[:], mybir.ActivationFunctionType.Exp)

            # ---- x DMA (8 chunks so early projections can start early) ----
            x_sb = apool.tile([C, N], BF16, tag="x")
            for g in range(N // 512):
                nc.gpsimd.dma_start(x_sb[:, 512 * g : 512 * (g + 1)], x[g])

            q_all = apool.tile([C, NQ], BF16, tag="q")    # [4h*32c', i]
            k_all = apool.tile([C, N], BF16, tag="k")     # [4h*32c', j]
            vT_all = apool.tile([C, N], BF16, tag="vT")   # chunk J cols J*128.. : [j, 4h*32d]

            # ---- projection units, streamed through the psum ring ----
            def emit_proj_batch(units):
                width = 512 * len(units)
                ps = pring.tile([128, width], F32, tag="ring", name="proj_ps")
                for u, (kind, g) in enumerate(units):
                    sl = slice(512 * u, 512 * (u + 1))
                    if kind == "q":
                        nc.tensor.matmul(
                            ps[:, sl],
                            lhsT=wq_sb[:],
                            rhs=x_sb[:, 512 * g : 512 * (g + 1)],
                            start=True,
                            stop=True,
                        )
                        nc.vector.tensor_copy(q_all[:, 512 * g : 512 * (g + 1)], ps[:, sl])
                    elif kind == "k":
                        nc.tensor.matmul(
                            ps[:, sl],
                            lhsT=wk_sb[:],
                            rhs=x_sb[:, 512 * g : 512 * (g + 1)],
                            start=True,
                            stop=True,
                        )
                        nc.vector.tensor_copy(k_all[:, 512 * g : 512 * (g + 1)], ps[:, sl])
                    else:  # v^T: per 128-pixel chunk, out[j, o'] = x^T wv
                        for c4 in range(4):
                            J = 4 * g + c4
                            nc.tensor.matmul(
                                ps[:, 512 * u + 128 * c4 : 512 * u + 128 * (c4 + 1)],
                                lhsT=x_sb[:, 128 * J : 128 * (J + 1)],
                                rhs=wv_sb[:],
                                start=True,
                                stop=True,
                            )
                        nc.vector.tensor_copy(vT_all[:, 512 * g : 512 * (g + 1)], ps[:, sl])

            # ---- attention stream ----
            # deferral scheduler: actions run at the start of their due body,
            # in insertion order (bodies past the window list keep flushing)
            acc_tiles = {}
            pending = []  # (due_body, fn)

            def schedule(due, fn):
                pending.append((due, fn))

            def flush(w):
                i = 0
                while i < len(pending):
                    due, fn = pending[i]
                    if due <= w:
                        pending.pop(i)
                        fn()
                    else:
                        i += 1

            half_sums = {}  # (I, h) -> (pTw, col) of the even-J partner

            def emit_pv(slots, pTw, w):
                for k, (I, J, h) in enumerate(slots):
                    if I not in acc_tiles:
                        acc_tiles[I] = (
                            pacc.tile([128, I_CHUNK], F32, tag="yT", name="yT"),
                            pacc.tile([128, I_CHUNK], F32, tag="sums", name="sums"),
                        )
                    nc.tensor.matmul(
                        acc_tiles[I][0][32 * h : 32 * h + 32, :],
                        lhsT=vT_all[:, 128 * J + 32 * h : 128 * J + 32 * h + 32],
                        rhs=pTw[:, 512 * k : 512 * (k + 1)],
                        start=(J == 0),
                        stop=(J == N_J - 1),
                        tile_position=(0, 32 * h),
                    )
                # denominators: pair adjacent j-chunks' P on VectorE, then one
                # ones-matmul per pair (emitted one body later so the DVE add
                # is never on the PE/ACT critical path) — halves the PE's
                # sums-stream cost
                for k, (I, J, h) in enumerate(slots):
                    if J % 2 == 0:
                        half_sums[(I, h)] = (pTw, k)
                        continue
                    ppTw, pk = half_sums.pop((I, h))
                    pair = epool.tile([128, I_CHUNK], BF16, tag="pair", name="pair", bufs=4)
                    nc.vector.tensor_tensor(
                        pair[:],
                        ppTw[:, 512 * pk : 512 * (pk + 1)],
                        pTw[:, 512 * k : 512 * (k + 1)],
                        mybir.AluOpType.add,
                    )

                    def sums_mm(I=I, J=J, h=h, pair=pair):
                        nc.tensor.matmul(
                            acc_tiles[I][1][32 * h : 32 * h + 32, :],
                            lhsT=ones32[:],
                            rhs=pair[:],
                            start=(J == 1),
                            stop=(J == N_J - 1),
                            tile_position=(0, 32 * h),
                        )
                        if (J, h) == (N_J - 1, HEADS - 1):
                            ynorm = emit_epilogue1(I)
                            schedule(w + 1, lambda: emit_epilogue2(I, ynorm))

                    # the chunk's last pairs skip the one-window deferral:
                    # the epilogue (and the next chunk's accumulator banks)
                    # wait on them, and there is no later exp to protect
                    schedule(w if J == N_J - 1 else w + 1, sums_mm)

            def emit_epilogue1(I):
                yT, sums = acc_tiles.pop(I)
                s_r = epool.tile([128, I_CHUNK], F32, tag="s_r")
                nc.vector.reciprocal_approx_fast(s_r[:], sums[:])
                ynorm = epool.tile([128, I_CHUNK], BF16, tag="ynorm")
                nc.vector.tensor_tensor(ynorm[:], yT[:], s_r[:], mybir.AluOpType.mult)
                return ynorm

            def emit_epilogue2(I, ynorm):
                isl = slice(I_CHUNK * I, I_CHUNK * (I + 1))
                # outproj borrows the sums bank (already freed by the recip)
                op = pacc.tile([128, I_CHUNK], F32, tag="sums", name="op")
                nc.tensor.matmul(op[:], lhsT=wo_sb[:], rhs=ynorm[:], start=True, stop=True)
                out_sb = epool.tile([128, I_CHUNK], F32, tag="out_sb")
                nc.vector.tensor_scalar_add(out_sb[:], op[:], bo_sb[:, :])
                nc.sync.dma_start(out[:, isl], out_sb[:])

            def emit_window(slots, w):
                width = 512 * len(slots)
                simw = pring.tile([128, width], F32, tag="ring", name="simw")
                for k, (I, J, h) in enumerate(slots):
                    nc.tensor.matmul(
                        simw[:, 512 * k : 512 * (k + 1)],
                        lhsT=k_all[32 * h : 32 * h + 32, 128 * J : 128 * (J + 1)],
                        rhs=q_all[32 * h : 32 * h + 32, I_CHUNK * I : I_CHUNK * (I + 1)],
                        start=True,
                        stop=True,
                        tile_position=(32 * h, 0),
                    )
                pTw = ptpool.tile([128, width], BF16, tag="pT", name="pTw")
                nc.scalar.activation(
                    pTw[:], simw[:], mybir.ActivationFunctionType.Exp, scale=SCALE
                )
                flush(w)
                schedule(w + 1, lambda: emit_pv(slots, pTw, w + 1))

            # window list: per i-chunk, 42 windows of 3 head-sims + 1 of 2
            windows = []
            for I in range(N_I):
                slots = [(I, t // 4, t % 4) for t in range(N_J * HEADS)]
                windows.extend(
                    slots[3 * w : 3 * w + 3] for w in range((len(slots) + 2) // 3)
                )

            # projections: prime the ring with what window 0 needs, then
            # spread the rest as single units on a just-in-time schedule so
            # they never pile PE work into the ramp windows
            emit_proj_batch([("k", 0), ("q", 0)])
            emit_proj_batch([("k", 1), ("v", 0)])
            late_units = [
                ("v", 1), ("k", 2), ("v", 2), ("k", 3), ("v", 3), ("k", 4),
                ("v", 4), ("k", 5), ("v", 5), ("k", 6), ("v", 6), ("k", 7),
                ("v", 7), ("q", 1), ("q", 2), ("q", 3),
            ]
            for i, unit in enumerate(late_units):
                schedule(2 + 2 * i, lambda unit=unit: emit_proj_batch([unit]))
            for w, slots in enumerate(windows):
                emit_window(slots, w)
            w = len(windows)
            while pending:
                flush(w)
                w += 1

    nc.compile()
    return nc


def kernel(x, w_qkv, w_out, b_out, _trace=False):
    if "nc" not in _NC_CACHE:
        _NC_CACHE["nc"] = _build_nc()
    nc = _NC_CACHE["nc"]

    x = np.asarray(x, dtype=np.float32).reshape(B, C, N)
    w_qkv = np.asarray(w_qkv, dtype=np.float32)
    w_out = np.asarray(w_out, dtype=np.float32)
    b_out = np.asarray(b_out, dtype=np.float32)

    wq = np.ascontiguousarray(w_qkv[0:C].T).astype(NPBF16)
    wk = np.ascontiguousarray(w_qkv[C : 2 * C].T).astype(NPBF16)
    wv = np.ascontiguousarray(w_qkv[2 * C : 3 * C].T).astype(NPBF16)
    wo = np.ascontiguousarray(w_out.T).astype(NPBF16)
    bo = np.ascontiguousarray(b_out.reshape(C, 1))

    in_maps = []
    for core in range(8):
        b, half = core >> 1, core & 1
        xb = x[b]
        if half:
            xb = np.concatenate([xb[:, NQ:], xb[:, :NQ]], axis=1)
        xb_c = np.ascontiguousarray(
            xb.reshape(C, N // 512, 512).transpose(1, 0, 2)
        ).astype(NPBF16)
        in_maps.append(
            {
                "x": xb_c,
                "wq": wq,
                "wk": wk,
                "wv": wv,
                "wo": wo,
                "bo": bo,
            }
        )

    res = run_bass_kernel_spmd(nc, in_maps, list(range(8)), trace=_trace)

    full = np.empty((B, C, N), np.float32)
    for core in range(8):
        b, half = core >> 1, core & 1
        full[b][:, NQ * half : NQ * (half + 1)] = res.results[core]["out"]
    out = full.reshape(B, C, 64, 64)
    if _trace:
        return out, res
    return out
